# revision 13
# baseline (speedup 1.0000x reference)
"""GCN classifier (2x GCNConv + mean-pool + linear) on 8 Trainium2 NeuronCores.

Sharding: nodes (and their incident edges, partitioned by edge dst) are sharded
across the 8 cores; the small 128x128 weights are replicated; boundary node
features are exchanged via AllGather (split into 4 quarter-AGs so edge gathers
start while later quarters are still in flight).

v3 structure:
  - embedding+xw1 fused: host premultiplies the (tiny) embedding tables by W1
    and prebuilds transposed one-hot planes; the device does 3 matmuls per
    128-node tile straight into y1 = dinv*(emb@W1). No gathers, no transposes.
  - per-conv AllGather split into 4 equal src-quarters; gather queue q reads
    quarter q, so descgen/DMA for quarter 0 starts after 1/4 of the AG wire
    time. y_slice quarter DMAs depend only on that quarter's xw tiles.
  - edge gather indices live in ONE persistent SBUF tile shared by both convs;
    pad slots use idx=-1 (ucode trims trailing -1 runs; mid-stream -1 gathers
    garbage that the one-hot zeroes).
  - scatter per dst-block: one-hot(dst) built on DVE, PE matmul-accumulates
    4*Cq chunks into PSUM (4 banks), epilogue adds self-loop/bias, scales,
    relu.
  - conv2 xw uses HWDGE dma_start_transpose for h^T (no PE transposes).
  - pool accumulates transposed (lhsT=h tile), applies Wlin BEFORE the
    AllReduce so the AR moves [G,10] not [G,128]; mean-divide + blin after.
"""

import math
import os
import sys

sys.path.insert(0, "/opt/trn_rl_repo")

import ml_dtypes
import numpy as np

import concourse.bass as bass
import concourse.mybir as mybir
import concourse.tile as tile
from concourse import bacc
from concourse.bass_utils import run_bass_kernel_spmd
from concourse.masks import make_identity

BF16 = mybir.dt.bfloat16
F32 = mybir.dt.float32
I16 = mybir.dt.int16
I32 = mybir.dt.int32
NP_BF16 = ml_dtypes.bfloat16

P = 128
NCORES = 8

# problem sizes (hardcoded per the harness contract)
CFG = dict(N=100_000, E=1_600_000, G=1024, F=128, NCLS=10)

GB = int(os.environ.get("K_GB", "4"))  # dst blocks per gather group
MSGB = int(os.environ.get("K_MSGB", "8"))  # msg pool buffers
EGB = int(os.environ.get("K_EGB", "7"))  # embed tiles per streamed chunk
SP = os.environ.get("K_SP", "0") == "1"  # single_packet for dma_gather
USE_DMAT = os.environ.get("K_DMAT", "0") == "1"  # dma_start_transpose for conv2 xw


def _plan(cfg):
    N, G = cfg["N"], cfg["G"]
    p = {}
    p["NPC"] = N // NCORES  # nodes per core
    p["NB"] = math.ceil(p["NPC"] / P)  # 128-node blocks per core
    p["NBP"] = p["NB"] * P
    p["QR"] = p["NPC"] // 4  # src quarter rows per core (3125)
    assert p["NPC"] % 4 == 0
    p["RROWS"] = NCORES * p["QR"]  # gather region rows (25000 < 32768)
    assert p["RROWS"] < 32768
    p["groups"] = [
        list(range(g, min(g + GB, p["NB"]))) for g in range(0, p["NB"], GB)
    ]
    p["NGT"] = G // P  # graph tiles
    assert G % P == 0
    return p


def _wrap16(idx_flat):
    """int16 index list -> [128, n/16] wrapped in 16 partitions, replicated 8x."""
    return np.tile(idx_flat.reshape(-1, 16).T, (8, 1))


def _prep_host(x, edge_index, batch, cfg, inputs=None):
    """Integer/index preprocessing + per-core metadata. Returns (per_core, Cq, pl)."""
    pl = _plan(cfg)
    N, G, F = cfg["N"], cfg["G"], cfg["F"]
    NPC, NB, NBP, QR = pl["NPC"], pl["NB"], pl["NBP"], pl["QR"]

    src = np.asarray(edge_index[0], np.int64)
    dst = np.asarray(edge_index[1], np.int64)
    batch = np.asarray(batch, np.int64)
    x = np.asarray(x, np.int64)

    deg_p1 = (np.bincount(dst, minlength=N) + 1).astype(np.float32)
    cnt = np.maximum(np.bincount(batch, minlength=G), 1).astype(np.float32)
    cnt_pt = cnt.reshape(pl["NGT"], P).T.copy()  # [P, NGT]

    core_of = dst // NPC
    per_core_edge = []  # (sorted sloc, sorted w, counts per cell)
    Cq = 1
    for k in range(NCORES):
        m = core_of == k
        s_k, d_k = src[m], dst[m] - k * NPC
        blk = d_k >> 7
        s_core = s_k // NPC
        s_local = s_k % NPC
        q = s_local // QR  # src quarter 0..3
        key = (blk * 4 + q).astype(np.int64)
        sloc_all = s_core * QR + (s_local % QR)
        # sort by cell, then by src row within cell (HBM locality)
        order = np.lexsort((sloc_all, key))
        counts = np.bincount(key, minlength=NB * 4)
        Cq = max(Cq, math.ceil(counts.max() / P))
        sloc = sloc_all.astype(np.int16)[order]
        w = (d_k & 127).astype(np.float32)[order]
        per_core_edge.append((sloc, w, counts))

    # tabw = [shape;color;pos] @ W1 premultiplied on host, split into 3
    # 128-row planes matching the one-hot planes below
    tab_cat = np.concatenate(
        [np.asarray(inputs["shape_tab"], np.float32),
         np.asarray(inputs["color_tab"], np.float32),
         np.asarray(inputs["pos_tab"], np.float32)], 0)  # [288, F]
    tabw_cat = tab_cat @ np.asarray(inputs["W1"], np.float32)  # [288, F]
    tabw = np.zeros((3 * P, F), np.float32)
    tabw[: 2 * P] = tabw_cat[: 2 * P]
    tabw[2 * P : 2 * P + 32] = tabw_cat[2 * P :]
    tabw = tabw.astype(NP_BF16)

    per_core = []
    for k in range(NCORES):
        sloc, w, counts = per_core_edge[k]
        cap = Cq * P
        src_pad = np.zeros((NB * 4, cap), np.int16)
        dst_pad = np.full((NB * 4, cap), -1.0, np.float32)
        starts = np.concatenate([[0], np.cumsum(counts)])
        for cell in range(NB * 4):
            c0, c1 = starts[cell], starts[cell + 1]
            n = c1 - c0
            if n:
                src_pad[cell, :n] = sloc[c0:c1]
                dst_pad[cell, :n] = w[c0:c1]

        idx_cols, dst_cols = [], []
        for blocks in pl["groups"]:
            for q in range(4):
                cells = [b * 4 + q for b in blocks]
                flat = src_pad[cells].reshape(-1).copy()
                if os.environ.get("K_NEG1", "0") == "1":
                    # NB: -1 trailing-trim HANGS the device (descriptor-count
                    # mismatch between decode-side ring bookkeeping and the
                    # trimmed Q7 push) - keep pads as idx 0.
                    dflat = dst_pad[cells].reshape(-1)
                    # trailing pad run -> idx -1 (ucode trims it: fewer descs)
                    nz = np.nonzero(dflat != -1.0)[0]
                    last = nz[-1] + 1 if len(nz) else 0
                    flat[last:] = -1
                idx_cols.append(_wrap16(flat))
            for b in blocks:
                # block-major: the 4*Cq chunk columns of block b, (q, cc) order
                cells = [b * 4 + q for q in range(4)]
                dst_cols.append(dst_pad[cells].reshape(-1, P).T)
        edge_idx = np.concatenate(idx_cols, 1)  # [128, TOTCOL] i16
        dstc = np.concatenate(dst_cols, 1).astype(NP_BF16)  # [128, NCH]

        # degree (layout [p, c] = local node c*128+p), pad nodes -> deg+1 = 1
        dp = np.ones(NBP, np.float32)
        dp[:NPC] = deg_p1[k * NPC : (k + 1) * NPC]
        dp = dp.reshape(NB, P).T.copy()

        # pool metadata
        bl = batch[k * NPC : (k + 1) * NPC]
        gbase = int(bl[0])
        gspan = int(bl[-1]) - gbase + 1
        assert gspan <= 2 * P, f"core {k} graph span {gspan} > 256"
        blf = np.full(NBP, -1.0, np.float32)
        blf[:NPC] = (bl - gbase).astype(np.float32)
        bl0 = blf.reshape(NB, P).T.astype(np.float32)
        bl1 = (blf - P).reshape(NB, P).T.astype(np.float32)
        gidx = np.zeros((P, 2), np.int32)
        for h in range(2):
            v = gbase + h * P + np.arange(P)
            v = np.where(v < G, v, G + (v % 8))
            gidx[:, h] = v

        # transposed one-hot planes [P, NB*3*F] bf16:
        # plane0 rows: 0-15 shape, 16-31 color, 32-127 pos 0..95
        # plane1 rows: pos 96..223;  plane2 rows: 0-31 pos 224..255, rest 0
        xi = np.zeros((NBP, 3), np.int64)
        xi[:NPC] = x[k * NPC : (k + 1) * NPC]
        r = np.arange(P)
        ohT = np.zeros((P, NB, 3, F), NP_BF16)
        for t in range(NB):
            xt = xi[t * P : (t + 1) * P]
            ohT[:, t, 0, :] = (
                (r[:, None] == xt[None, :, 0]) & (r[:, None] < 16)
                | (r[:, None] - 16 == xt[None, :, 1]) & (r[:, None] >= 16) & (r[:, None] < 32)
                | (r[:, None] - 32 == xt[None, :, 2]) & (r[:, None] >= 32)
            ).astype(NP_BF16)
            ohT[:, t, 1, :] = (r[:, None] + 96 == xt[None, :, 2]).astype(NP_BF16)
            ohT[:, t, 2, :] = (
                (r[:, None] + 224 == xt[None, :, 2]) & (r[:, None] < 32)
            ).astype(NP_BF16)
        ohT = ohT.reshape(P, NB * 3 * F)

        per_core.append(
            dict(
                deg_p1=dp,
                bl0=bl0,
                bl1=bl1,
                gidx=gidx,
                cnt=cnt_pt,
                tabw=tabw,
                ohT=ohT,
                edge_idx=edge_idx,
                dst_cols=dstc,
            )
        )
    return per_core, Cq, pl


def _build(cfg, Cq, pl, totcol, nch):
    """Build the SPMD Bass program (one NEFF for all 8 cores)."""
    PHASES = int(os.environ.get("K_PHASES", "9"))  # 1=embed 2=+conv1 3=+conv2 9=all
    SUB = int(os.environ.get("K_SUB", "9"))  # 1=xw+AG 2=+gathers 3=+onehot 4=+matmul/epi
    NOAG = int(os.environ.get("K_NOAG", "0"))  # 1: replace AllGather with local copies
    N, G, F, NCLS = cfg["N"], cfg["G"], cfg["F"], cfg["NCLS"]
    NPC, NB, QR, RROWS, NGT = pl["NPC"], pl["NB"], pl["QR"], pl["RROWS"], pl["NGT"]
    groups = pl["groups"]

    nc = bacc.Bacc("TRN2", num_devices=NCORES, num_swdge_queues=4)
    RG = [list(range(NCORES))]

    # ---- I/O ----
    tabwd = nc.dram_tensor("tabw", [3 * P, F], BF16, kind="ExternalInput")
    ohTd = nc.dram_tensor("ohT", [P, NB * 3 * F], BF16, kind="ExternalInput")
    W2d = nc.dram_tensor("W2", [F, F], F32, kind="ExternalInput")
    b1d = nc.dram_tensor("b1", [1, F], F32, kind="ExternalInput")
    b2d = nc.dram_tensor("b2", [1, F], F32, kind="ExternalInput")
    Wld = nc.dram_tensor("Wlin", [F, NCLS], F32, kind="ExternalInput")
    bld = nc.dram_tensor("blin", [1, NCLS], F32, kind="ExternalInput")
    degd = nc.dram_tensor("deg_p1", [P, NB], F32, kind="ExternalInput")
    bl0d = nc.dram_tensor("bl0", [P, NB], F32, kind="ExternalInput")
    bl1d = nc.dram_tensor("bl1", [P, NB], F32, kind="ExternalInput")
    gixd = nc.dram_tensor("gidx", [P, 2], I32, kind="ExternalInput")
    cntd = nc.dram_tensor("cnt", [P, NGT], F32, kind="ExternalInput")
    xixd = nc.dram_tensor("edge_idx", [P, totcol], I16, kind="ExternalInput")
    dcd = nc.dram_tensor("dst_cols", [P, nch], BF16, kind="ExternalInput")
    outd = nc.dram_tensor("out", [G, NCLS], F32, kind="ExternalOutput")

    with tile.TileContext(nc) as tc:
        import contextlib

        ctx = contextlib.ExitStack()
        persist = ctx.enter_context(tc.tile_pool(name="persist", bufs=1))
        dramp = ctx.enter_context(tc.tile_pool(name="dramp", bufs=1, space="DRAM"))
        xw_pool = ctx.enter_context(tc.tile_pool(name="xw", bufs=2, space="PSUM"))
        acc_pool = ctx.enter_context(tc.tile_pool(name="acc", bufs=4, space="PSUM"))
        pacc_pool = ctx.enter_context(tc.tile_pool(name="pacc", bufs=1, space="PSUM"))
        sb_pool = ctx.enter_context(tc.tile_pool(name="work", bufs=3))
        msg_pool = ctx.enter_context(tc.tile_pool(name="msg", bufs=MSGB))
        oh_pool = ctx.enter_context(tc.tile_pool(name="oh", bufs=3))
        emb_pool = ctx.enter_context(tc.tile_pool(name="emb", bufs=3))
        craw = ctx.enter_context(tc.tile_pool(name="craw", bufs=1))

        def T(shape, dt, space=None, addr_space="Local", name=None):
            pool = dramp if space == "DRAM" else persist
            return pool.tile(shape, dt, tag=name, name=name, addr_space=addr_space)

        # ---- internal DRAM ----
        y_slice = [
            T([NPC, F], BF16, space="DRAM", name=f"y_slice{c}") for c in range(2)
        ]
        y_fq = [
            [T([RROWS, F], BF16, space="DRAM",
               addr_space="Local" if NOAG else "Shared", name=f"y_fq{c}_{q}")
             for q in range(4)]
            for c in range(2)
        ]
        dram_lin = T([G + 8, NCLS], F32, space="DRAM", name="dram_lin")
        ar_lin = T([G + 8, NCLS], F32, space="DRAM", addr_space="Shared",
                   name="ar_lin")

        # ---- persistent SBUF ----
        hA = T([P, NB * F], BF16, name="hA")  # conv2 output (node-major)
        hB = T([P, NB * F], BF16, name="hB")  # conv1 output (node-major)
        y_nm = T([P, NB * F], BF16, name="y_nm")
        dstc_sb = T([P, nch], BF16, name="dstc_sb")
        nc.sync.dma_start(out=dstc_sb[:], in_=dcd[:])
        eix_sb = T([P, totcol], I16, name="eix_sb")
        nc.sync.dma_start(out=eix_sb[:], in_=xixd[:])

        # constants
        iota_i = craw.tile([P, P], I32, tag="iota_i", name="iota_i")
        nc.gpsimd.iota(iota_i[:], pattern=[[1, P]], base=0, channel_multiplier=0)
        iota_bf = T([P, P], BF16, name="iota_bf")
        nc.vector.tensor_copy(iota_bf[:], iota_i[:])
        iota_f = T([P, P], F32, name="iota_f")
        nc.vector.tensor_copy(iota_f[:], iota_i[:])
        ones_row = T([1, P], F32, name="ones_row")
        nc.vector.memset(ones_row[:], 1.0)
        id_bf = T([P, P], BF16, name="id_bf")
        make_identity(nc, id_bf[:])

        def load_cast(name, dram, shape, dt_in, dt_out):
            t = T(shape, dt_out, name=name)
            if dt_out == dt_in:
                nc.sync.dma_start(out=t[:], in_=dram[:])
            else:
                # NB: SWDGE cast-DMA + indirect_dma in one program crashes the
                # device (observed NRT_EXEC_UNIT_UNRECOVERABLE) - cast on DVE.
                raw = craw.tile(shape, dt_in, tag=name + "_r", name=name + "_r")
                nc.sync.dma_start(out=raw[:], in_=dram[:])
                nc.vector.tensor_copy(t[:], raw[:])
            return t

        # tabw [3*P, F] in DRAM -> SBUF [P, 3*F] (plane-major in free dim)
        tabw_sb = T([P, 3 * F], BF16, name="tabw_sb")
        nc.sync.dma_start(
            out=tabw_sb[:].rearrange("p (c f) -> p c f", f=F),
            in_=tabwd[:].rearrange("(c p) f -> p c f", p=P),
        )
        W2c = load_cast("W2", W2d, [F, F], F32, BF16)
        bc = [
            load_cast("b1", b1d, [1, F], F32, F32),
            load_cast("b2", b2d, [1, F], F32, F32),
        ]
        Wl_sb = load_cast("Wl", Wld, [F, NCLS], F32, F32)
        bl_sb = load_cast("bl", bld, [1, NCLS], F32, F32)
        bl0_sb = load_cast("bl0", bl0d, [P, NB], F32, F32)
        bl1_sb = load_cast("bl1", bl1d, [P, NB], F32, F32)
        cnt_sb = load_cast("cnt", cntd, [P, NGT], F32, F32)
        gix_sb = load_cast("gix", gixd, [P, 2], I32, I32)

        # dinv = 1/sqrt(deg+1); sq = sqrt(deg+1)
        deg_sb = craw.tile([P, NB], F32, tag="deg_sb", name="deg_sb")
        nc.sync.dma_start(out=deg_sb[:], in_=degd[:])
        sq_sb = T([P, NB], F32, name="sq_sb")
        nc.scalar.sqrt(sq_sb[:], deg_sb[:])
        dinv = T([P, NB], F32, name="dinv")
        nc.vector.reciprocal(dinv[:], sq_sb[:])

        # blin broadcast down partitions (rank-1 via PE)
        blb_ps = xw_pool.tile([P, NCLS], F32, tag="xw")
        nc.tensor.matmul(blb_ps[:], lhsT=ones_row[:], rhs=bl_sb[:],
                         start=True, stop=True)
        blin_b = T([P, NCLS], F32, name="blin_b")
        nc.scalar.copy(blin_b[:], blb_ps[:])

        # zero dram_lin (pool scatter target) early
        zsb = craw.tile([P, NCLS * NGT], F32, tag="zsb", name="zsb")
        nc.vector.memset(zsb[:], 0.0)
        nc.sync.dma_start(
            out=dram_lin[:G, :].rearrange("(c p) f -> p c f", p=P),
            in_=zsb[:, : NGT * NCLS].rearrange("p (c f) -> p c f", f=NCLS),
        )
        nc.sync.dma_start(out=dram_lin[G:, :], in_=zsb[:8, :NCLS])

        def dma_node_rows(dst_dram, r0, r1):
            """DMA y_nm node rows [r0, r1) to dst_dram[r0:r1] (node-major)."""
            b0, o0 = divmod(r0, P)
            if o0:
                take = min(r1, (b0 + 1) * P) - r0
                nc.sync.dma_start(
                    out=dst_dram[r0 : r0 + take, :],
                    in_=y_nm[o0 : o0 + take, b0 * F : (b0 + 1) * F],
                )
                r0 += take
                b0 += 1
            nfull = (r1 - r0) // P
            if nfull:
                nc.sync.dma_start(
                    out=dst_dram[r0 : r0 + nfull * P, :].rearrange(
                        "(c p) f -> p c f", p=P),
                    in_=y_nm[:, b0 * F : (b0 + nfull) * F].rearrange(
                        "p (c f) -> p c f", f=F),
                )
                r0 += nfull * P
                b0 += nfull
            if r1 > r0:
                nc.sync.dma_start(
                    out=dst_dram[r0:r1, :],
                    in_=y_nm[: r1 - r0, b0 * F : (b0 + 1) * F],
                )

        def emit_ag(conv):
            for q in range(4):
                r0, r1 = q * QR, (q + 1) * QR
                dma_node_rows(y_slice[conv], r0, r1)
                if NOAG:
                    for kk in range(NCORES):
                        nc.sync.dma_start(
                            out=y_fq[conv][q][kk * QR : (kk + 1) * QR, :],
                            in_=y_slice[conv][r0:r1, :],
                        )
                else:
                    nc.gpsimd.collective_compute(
                        "AllGather",
                        mybir.AluOpType.bypass,
                        replica_groups=RG,
                        ins=[y_slice[conv][r0:r1, :]],
                        outs=[y_fq[conv][q][:]],
                    )

        # ---------------- embedding + xw1 fused -> y_nm ----------------
        ESC = math.ceil(NB / EGB)
        for s in range(ESC):
            t0 = s * EGB
            t1 = min(t0 + EGB, NB)
            ch = emb_pool.tile([P, EGB * 3 * F], BF16, tag="embch")
            nc.sync.dma_start(
                out=ch[:, : (t1 - t0) * 3 * F],
                in_=ohTd[:, t0 * 3 * F : t1 * 3 * F],
            )
            for t in range(t0, t1):
                base = (t - t0) * 3 * F
                xw = xw_pool.tile([P, P], F32, tag="xw")
                for j in range(3):
                    nc.tensor.matmul(
                        xw[:],
                        lhsT=ch[:, base + j * F : base + (j + 1) * F],
                        rhs=tabw_sb[:, j * F : (j + 1) * F],
                        start=(j == 0),
                        stop=(j == 2),
                    )
                nc.scalar.activation(
                    y_nm[:, t * F : (t + 1) * F],
                    xw[:],
                    mybir.ActivationFunctionType.Copy,
                    scale=dinv[:, t : t + 1],
                )
        emit_ag(0)

        # ---------------- two GCN convs ----------------
        for conv in range(min(2, max(0, PHASES - 1))):
            hout = hB if conv == 0 else hA

            # b_bcast[n, f] = b[f] replicated down partitions (rank-1 via PE)
            bb_ps = xw_pool.tile([P, P], F32, tag="xw")
            nc.tensor.matmul(bb_ps[:], lhsT=ones_row[:], rhs=bc[conv][:],
                             start=True, stop=True)
            b_bcast = craw.tile([P, P], F32, tag=f"b_bcast{conv}", name=f"b_bcast{conv}")
            nc.scalar.copy(b_bcast[:], bb_ps[:])

            # scatter phase
            ch_off = 0  # chunk offset into dst_cols
            col_off = 0  # column offset into edge_idx
            for blocks in (groups if SUB >= 2 else []):
                nblk = len(blocks)
                nch_q = nblk * Cq
                nidx = nch_q * P
                msgs = []
                for q in range(4):
                    msg = msg_pool.tile([P, GB * Cq, F], BF16, tag="msg")
                    nc.gpsimd.dma_gather(
                        out_ap=msg[:, :nch_q, :],
                        in_ap=y_fq[conv][q][:, :],
                        idxs_ap=eix_sb[:, col_off : col_off + nidx // 16],
                        num_idxs=nidx,
                        num_idxs_reg=nidx,
                        elem_size=F,
                        single_packet=SP,
                        queue_num=q,
                    )
                    col_off += nidx // 16
                    msgs.append(msg)
                if SUB < 3:
                    ch_off += 4 * nblk * Cq
                    continue
                for bi, b in enumerate(blocks):
                    oh = oh_pool.tile([P, 4 * Cq, P], BF16, tag="oh")
                    nc.vector.tensor_tensor(
                        out=oh[:],
                        in0=iota_bf[:].unsqueeze(1).broadcast_to([P, 4 * Cq, P]),
                        in1=dstc_sb[:, ch_off + bi * 4 * Cq : ch_off + (bi + 1) * 4 * Cq]
                        .unsqueeze(2)
                        .broadcast_to([P, 4 * Cq, P]),
                        op=mybir.AluOpType.is_equal,
                    )
                    if SUB < 4:
                        continue
                    acc = acc_pool.tile([P, P], F32, tag="acc")
                    j = 0
                    for q in range(4):
                        for cc in range(Cq):
                            nc.tensor.matmul(
                                acc[:],
                                lhsT=oh[:, q * Cq + cc, :],
                                rhs=msgs[q][:, bi * Cq + cc, :],
                                start=(j == 0),
                                stop=(j == 4 * Cq - 1),
                            )
                            j += 1
                    # bias: bb = b (x) rdinv (cancels the later *dinv); ACT op
                    bb = sb_pool.tile([P, P], F32, tag="bb")
                    nc.scalar.activation(
                        bb[:], b_bcast[:], mybir.ActivationFunctionType.Copy,
                        scale=sq_sb[:, b : b + 1],
                    )
                    hs = sb_pool.tile([P, P], F32, tag="ep")
                    nc.vector.tensor_tensor(
                        out=hs[:],
                        in0=acc[:],
                        in1=y_nm[:, b * F : (b + 1) * F],
                        op=mybir.AluOpType.add,
                    )
                    nc.vector.tensor_tensor(
                        out=hs[:], in0=hs[:], in1=bb[:], op=mybir.AluOpType.add,
                    )
                    nc.vector.tensor_tensor(
                        out=hs[:],
                        in0=hs[:],
                        in1=dinv[:, b : b + 1].to_broadcast([P, P]),
                        op=mybir.AluOpType.mult,
                    )
                    nc.scalar.activation(
                        hout[:, b * F : (b + 1) * F],
                        hs[:],
                        mybir.ActivationFunctionType.Relu,
                    )
                ch_off += 4 * nch_q

            # xw phase for conv2: y2 = dinv * (h1 @ W2), h^T via DMA transpose
            if conv == 0 and PHASES >= 3:
                for t in range(NB):
                    hT = sb_pool.tile([P, P], BF16, tag="hT")
                    if USE_DMAT:
                        nc.sync.dma_start_transpose(
                            hT[:], hB[:, t * F : (t + 1) * F])
                    else:
                        tp = xw_pool.tile([P, P], BF16, tag="xw")
                        nc.tensor.transpose(tp[:], hB[:, t * F : (t + 1) * F],
                                            id_bf[:])
                        nc.scalar.copy(hT[:], tp[:])
                    xw = xw_pool.tile([P, P], F32, tag="xw")
                    nc.tensor.matmul(xw[:], lhsT=hT[:], rhs=W2c[:],
                                     start=True, stop=True)
                    nc.scalar.activation(
                        y_nm[:, t * F : (t + 1) * F],
                        xw[:],
                        mybir.ActivationFunctionType.Copy,
                        scale=dinv[:, t : t + 1],
                    )
                emit_ag(1)

        # ---------------- global mean pool + linear ----------------
        do_pool = PHASES >= 9
        if do_pool:
            # paccT[h][f, g] = sum_n h[n, f] * onehot(graph[n] == g_h)
            pacc = [pacc_pool.tile([P, P], F32, tag=f"pacc{h}", name=f"pacc{h}")
                    for h in range(2)]
            bls = [bl0_sb, bl1_sb]
            for t in range(NB):
                for h in range(2):
                    oht = sb_pool.tile([P, P], BF16, tag="pooloh")
                    nc.vector.tensor_tensor(
                        out=oht[:],
                        in0=iota_f[:],
                        in1=bls[h][:, t : t + 1].to_broadcast([P, P]),
                        op=mybir.AluOpType.is_equal,
                    )
                    nc.tensor.matmul(
                        pacc[h][:],
                        lhsT=hA[:, t * F : (t + 1) * F],
                        rhs=oht[:],
                        start=(t == 0),
                        stop=(t == NB - 1),
                    )
            for h in range(2):
                pT = sb_pool.tile([P, P], F32, tag="pT")
                nc.vector.tensor_copy(pT[:], pacc[h][:])
                lin_ps = xw_pool.tile([P, NCLS], F32, tag="xw")
                nc.tensor.matmul(lin_ps[:], lhsT=pT[:], rhs=Wl_sb[:],
                                 start=True, stop=True)
                linc = sb_pool.tile([P, NCLS], F32, tag="linc")
                nc.vector.tensor_copy(linc[:], lin_ps[:])
                nc.gpsimd.indirect_dma_start(
                    out=dram_lin[:],
                    out_offset=bass.IndirectOffsetOnAxis(ap=gix_sb[:, h : h + 1], axis=0),
                    in_=linc[:],
                    in_offset=None,
                )
            nc.gpsimd.collective_compute(
                "AllReduce",
                mybir.AluOpType.add,
                replica_groups=RG,
                ins=[dram_lin[:]],
                outs=[ar_lin[:]],
            )
            recip = T([P, NGT], F32, name="recip")
            nc.vector.reciprocal(recip[:], cnt_sb[:])
            for t in range(NGT):
                art = sb_pool.tile([P, NCLS], F32, tag="art")
                nc.sync.dma_start(out=art[:], in_=ar_lin[t * P : (t + 1) * P, :])
                pooled = sb_pool.tile([P, NCLS], F32, tag="linc")
                nc.vector.tensor_tensor(
                    out=pooled[:],
                    in0=art[:],
                    in1=recip[:, t : t + 1].to_broadcast([P, NCLS]),
                    op=mybir.AluOpType.mult,
                )
                oute = sb_pool.tile([P, NCLS], F32, tag="oute")
                nc.vector.tensor_tensor(
                    out=oute[:], in0=pooled[:], in1=blin_b[:],
                    op=mybir.AluOpType.add,
                )
                nc.sync.dma_start(out=outd[t * P : (t + 1) * P, :], in_=oute[:])

        else:
            dummy = sb_pool.tile([P, NCLS], F32, tag="oute", name="dummy")
            nc.vector.memset(dummy[:], 0.0)
            for t in range(NGT):
                nc.sync.dma_start(out=outd[t * P : (t + 1) * P, :], in_=dummy[:])

        ctx.close()
    nc.compile()
    return nc


_CACHE = {}


def _get_nc(cfg, Cq, pl, totcol, nch):
    key = (tuple(sorted(cfg.items())), Cq, totcol, nch, GB, MSGB, EGB, SP, USE_DMAT)
    if key not in _CACHE:
        _CACHE[key] = _build(cfg, Cq, pl, totcol, nch)
    return _CACHE[key]


def run(inputs, cfg, trace=False):
    x = np.asarray(inputs["x"])
    per_core, Cq, pl = _prep_host(x, np.asarray(inputs["edge_index"]),
                                  np.asarray(inputs["batch"]), cfg, inputs)
    totcol = per_core[0]["edge_idx"].shape[1]
    nch = per_core[0]["dst_cols"].shape[1]
    nc = _get_nc(cfg, Cq, pl, totcol, nch)

    shared = dict(
        W2=np.asarray(inputs["W2"], np.float32),
        b1=np.asarray(inputs["b1"], np.float32).reshape(1, -1),
        b2=np.asarray(inputs["b2"], np.float32).reshape(1, -1),
        Wlin=np.asarray(inputs["Wlin"], np.float32),
        blin=np.asarray(inputs["blin"], np.float32).reshape(1, -1),
    )
    in_maps = [{**shared, **per_core[k]} for k in range(NCORES)]
    res = run_bass_kernel_spmd(nc, in_maps, list(range(NCORES)), trace=trace)
    out = np.asarray(res.results[0]["out"], np.float32)
    return out, res


def kernel(**inputs) -> np.ndarray:
    out, _ = run(inputs, CFG)
    return out


# revision 17
# speedup vs baseline: 1.0114x; 1.0114x over previous
"""GCN classifier (2x GCNConv + mean-pool + linear) on 8 Trainium2 NeuronCores.

Sharding: nodes (and their incident edges, partitioned by edge dst) are sharded
across the 8 cores; the small 128x128 weights are replicated; boundary node
features are exchanged via AllGather (split into 4 quarter-AGs so edge gathers
start while later quarters are still in flight).

v3 structure:
  - embedding+xw1 fused: host premultiplies the (tiny) embedding tables by W1
    and prebuilds transposed one-hot planes; the device does 3 matmuls per
    128-node tile straight into y1 = dinv*(emb@W1). No gathers, no transposes.
  - per-conv AllGather split into 4 equal src-quarters; gather queue q reads
    quarter q, so descgen/DMA for quarter 0 starts after 1/4 of the AG wire
    time. y_slice quarter DMAs depend only on that quarter's xw tiles.
  - edge gather indices live in ONE persistent SBUF tile shared by both convs;
    pad slots use idx=-1 (ucode trims trailing -1 runs; mid-stream -1 gathers
    garbage that the one-hot zeroes).
  - scatter per dst-block: one-hot(dst) built on DVE, PE matmul-accumulates
    4*Cq chunks into PSUM (4 banks), epilogue adds self-loop/bias, scales,
    relu.
  - conv2 xw uses HWDGE dma_start_transpose for h^T (no PE transposes).
  - pool accumulates transposed (lhsT=h tile), applies Wlin BEFORE the
    AllReduce so the AR moves [G,10] not [G,128]; mean-divide + blin after.
"""

import math
import os
import sys

sys.path.insert(0, "/opt/trn_rl_repo")

import ml_dtypes
import numpy as np

import concourse.bass as bass
import concourse.mybir as mybir
import concourse.tile as tile
from concourse import bacc
from concourse.bass_utils import run_bass_kernel_spmd
from concourse.masks import make_identity

BF16 = mybir.dt.bfloat16
F32 = mybir.dt.float32
I16 = mybir.dt.int16
I32 = mybir.dt.int32
NP_BF16 = ml_dtypes.bfloat16

P = 128
NCORES = 8

# problem sizes (hardcoded per the harness contract)
CFG = dict(N=100_000, E=1_600_000, G=1024, F=128, NCLS=10)

GB = int(os.environ.get("K_GB", "4"))  # dst blocks per gather group
MSGB = int(os.environ.get("K_MSGB", "8"))  # msg pool buffers
EGB = int(os.environ.get("K_EGB", "7"))  # embed tiles per streamed chunk
SP = os.environ.get("K_SP", "0") == "1"  # single_packet for dma_gather
USE_DMAT = os.environ.get("K_DMAT", "0") == "1"  # dma_start_transpose for conv2 xw


def _plan(cfg):
    N, G = cfg["N"], cfg["G"]
    p = {}
    p["NPC"] = N // NCORES  # nodes per core
    p["NB"] = math.ceil(p["NPC"] / P)  # 128-node blocks per core
    p["NBP"] = p["NB"] * P
    p["QR"] = p["NPC"] // 4  # src quarter rows per core (3125)
    assert p["NPC"] % 4 == 0
    p["RROWS"] = NCORES * p["QR"]  # gather region rows (25000 < 32768)
    assert p["RROWS"] < 32768
    p["groups"] = [
        list(range(g, min(g + GB, p["NB"]))) for g in range(0, p["NB"], GB)
    ]
    p["NGT"] = G // P  # graph tiles
    assert G % P == 0
    return p


def _wrap16(idx_flat):
    """int16 index list -> [128, n/16] wrapped in 16 partitions, replicated 8x."""
    return np.tile(idx_flat.reshape(-1, 16).T, (8, 1))


def _prep_host(x, edge_index, batch, cfg, inputs=None):
    """Integer/index preprocessing + per-core metadata. Returns (per_core, Cq, pl)."""
    pl = _plan(cfg)
    N, G, F = cfg["N"], cfg["G"], cfg["F"]
    NPC, NB, NBP, QR = pl["NPC"], pl["NB"], pl["NBP"], pl["QR"]

    src = np.asarray(edge_index[0], np.int64)
    dst = np.asarray(edge_index[1], np.int64)
    batch = np.asarray(batch, np.int64)
    x = np.asarray(x, np.int64)

    deg_p1 = (np.bincount(dst, minlength=N) + 1).astype(np.float32)
    cnt = np.maximum(np.bincount(batch, minlength=G), 1).astype(np.float32)
    cnt_pt = cnt.reshape(pl["NGT"], P).T.copy()  # [P, NGT]

    core_of = dst // NPC
    per_core_edge = []  # (sorted sloc, sorted w, counts per cell)
    Cq = 1
    for k in range(NCORES):
        m = core_of == k
        s_k, d_k = src[m], dst[m] - k * NPC
        blk = d_k >> 7
        s_core = s_k // NPC
        s_local = s_k % NPC
        q = s_local // QR  # src quarter 0..3
        key = (blk * 4 + q).astype(np.int64)
        sloc_all = s_core * QR + (s_local % QR)
        # sort by cell, then by src row within cell (HBM locality)
        order = np.lexsort((sloc_all, key))
        counts = np.bincount(key, minlength=NB * 4)
        Cq = max(Cq, math.ceil(counts.max() / P))
        sloc = sloc_all.astype(np.int16)[order]
        w = (d_k & 127).astype(np.float32)[order]
        per_core_edge.append((sloc, w, counts))

    # tabw = [shape;color;pos] @ W1 premultiplied on host, split into 3
    # 128-row planes matching the one-hot planes below
    tab_cat = np.concatenate(
        [np.asarray(inputs["shape_tab"], np.float32),
         np.asarray(inputs["color_tab"], np.float32),
         np.asarray(inputs["pos_tab"], np.float32)], 0)  # [288, F]
    tabw_cat = tab_cat @ np.asarray(inputs["W1"], np.float32)  # [288, F]
    tabw = np.zeros((3 * P, F), np.float32)
    tabw[: 2 * P] = tabw_cat[: 2 * P]
    tabw[2 * P : 2 * P + 32] = tabw_cat[2 * P :]
    tabw = tabw.astype(NP_BF16)

    per_core = []
    for k in range(NCORES):
        sloc, w, counts = per_core_edge[k]
        cap = Cq * P
        src_pad = np.zeros((NB * 4, cap), np.int16)
        dst_pad = np.full((NB * 4, cap), -1.0, np.float32)
        starts = np.concatenate([[0], np.cumsum(counts)])
        for cell in range(NB * 4):
            c0, c1 = starts[cell], starts[cell + 1]
            n = c1 - c0
            if n:
                src_pad[cell, :n] = sloc[c0:c1]
                dst_pad[cell, :n] = w[c0:c1]

        idx_cols, dst_cols = [], []
        for blocks in pl["groups"]:
            for q in range(4):
                cells = [b * 4 + q for b in blocks]
                flat = src_pad[cells].reshape(-1).copy()
                if os.environ.get("K_NEG1", "0") == "1":
                    # NB: -1 trailing-trim HANGS the device (descriptor-count
                    # mismatch between decode-side ring bookkeeping and the
                    # trimmed Q7 push) - keep pads as idx 0.
                    dflat = dst_pad[cells].reshape(-1)
                    # trailing pad run -> idx -1 (ucode trims it: fewer descs)
                    nz = np.nonzero(dflat != -1.0)[0]
                    last = nz[-1] + 1 if len(nz) else 0
                    flat[last:] = -1
                idx_cols.append(_wrap16(flat))
            for b in blocks:
                # block-major: the 4*Cq chunk columns of block b, (q, cc) order
                cells = [b * 4 + q for q in range(4)]
                dst_cols.append(dst_pad[cells].reshape(-1, P).T)
        edge_idx = np.concatenate(idx_cols, 1)  # [128, TOTCOL] i16
        dstc = np.concatenate(dst_cols, 1).astype(NP_BF16)  # [128, NCH]

        # degree (layout [p, c] = local node c*128+p), pad nodes -> deg+1 = 1
        dp = np.ones(NBP, np.float32)
        dp[:NPC] = deg_p1[k * NPC : (k + 1) * NPC]
        dp = dp.reshape(NB, P).T.copy()

        # pool metadata
        bl = batch[k * NPC : (k + 1) * NPC]
        gbase = int(bl[0])
        gspan = int(bl[-1]) - gbase + 1
        assert gspan <= 2 * P, f"core {k} graph span {gspan} > 256"
        blf = np.full(NBP, -1.0, np.float32)
        blf[:NPC] = (bl - gbase).astype(np.float32)
        bl0 = blf.reshape(NB, P).T.astype(np.float32)
        bl1 = (blf - P).reshape(NB, P).T.astype(np.float32)
        gidx = np.zeros((P, 2), np.int32)
        for h in range(2):
            v = gbase + h * P + np.arange(P)
            v = np.where(v < G, v, G + (v % 8))
            gidx[:, h] = v

        # transposed one-hot planes [P, NB*3*F] bf16:
        # plane0 rows: 0-15 shape, 16-31 color, 32-127 pos 0..95
        # plane1 rows: pos 96..223;  plane2 rows: 0-31 pos 224..255, rest 0
        xi = np.zeros((NBP, 3), np.int64)
        xi[:NPC] = x[k * NPC : (k + 1) * NPC]
        r = np.arange(P)
        ohT = np.zeros((P, NB, 3, F), NP_BF16)
        for t in range(NB):
            xt = xi[t * P : (t + 1) * P]
            ohT[:, t, 0, :] = (
                (r[:, None] == xt[None, :, 0]) & (r[:, None] < 16)
                | (r[:, None] - 16 == xt[None, :, 1]) & (r[:, None] >= 16) & (r[:, None] < 32)
                | (r[:, None] - 32 == xt[None, :, 2]) & (r[:, None] >= 32)
            ).astype(NP_BF16)
            ohT[:, t, 1, :] = (r[:, None] + 96 == xt[None, :, 2]).astype(NP_BF16)
            ohT[:, t, 2, :] = (
                (r[:, None] + 224 == xt[None, :, 2]) & (r[:, None] < 32)
            ).astype(NP_BF16)
        ohT = ohT.reshape(P, NB * 3 * F)

        per_core.append(
            dict(
                deg_p1=dp,
                bl0=bl0,
                bl1=bl1,
                gidx=gidx,
                cnt=cnt_pt,
                tabw=tabw,
                ohT=ohT,
                edge_idx=edge_idx,
                dst_cols=dstc,
            )
        )
    return per_core, Cq, pl


def _build(cfg, Cq, pl, totcol, nch):
    """Build the SPMD Bass program (one NEFF for all 8 cores)."""
    PHASES = int(os.environ.get("K_PHASES", "9"))  # 1=embed 2=+conv1 3=+conv2 9=all
    SUB = int(os.environ.get("K_SUB", "9"))  # 1=xw+AG 2=+gathers 3=+onehot 4=+matmul/epi
    NOAG = int(os.environ.get("K_NOAG", "0"))  # 1: replace AllGather with local copies
    N, G, F, NCLS = cfg["N"], cfg["G"], cfg["F"], cfg["NCLS"]
    NPC, NB, QR, RROWS, NGT = pl["NPC"], pl["NB"], pl["QR"], pl["RROWS"], pl["NGT"]
    groups = pl["groups"]

    nc = bacc.Bacc("TRN2", num_devices=NCORES, num_swdge_queues=4)
    RG = [list(range(NCORES))]

    # ---- I/O ----
    tabwd = nc.dram_tensor("tabw", [3 * P, F], BF16, kind="ExternalInput")
    ohTd = nc.dram_tensor("ohT", [P, NB * 3 * F], BF16, kind="ExternalInput")
    W2d = nc.dram_tensor("W2", [F, F], F32, kind="ExternalInput")
    b1d = nc.dram_tensor("b1", [1, F], F32, kind="ExternalInput")
    b2d = nc.dram_tensor("b2", [1, F], F32, kind="ExternalInput")
    Wld = nc.dram_tensor("Wlin", [F, NCLS], F32, kind="ExternalInput")
    bld = nc.dram_tensor("blin", [1, NCLS], F32, kind="ExternalInput")
    degd = nc.dram_tensor("deg_p1", [P, NB], F32, kind="ExternalInput")
    bl0d = nc.dram_tensor("bl0", [P, NB], F32, kind="ExternalInput")
    bl1d = nc.dram_tensor("bl1", [P, NB], F32, kind="ExternalInput")
    gixd = nc.dram_tensor("gidx", [P, 2], I32, kind="ExternalInput")
    cntd = nc.dram_tensor("cnt", [P, NGT], F32, kind="ExternalInput")
    xixd = nc.dram_tensor("edge_idx", [P, totcol], I16, kind="ExternalInput")
    dcd = nc.dram_tensor("dst_cols", [P, nch], BF16, kind="ExternalInput")
    outd = nc.dram_tensor("out", [G, NCLS], F32, kind="ExternalOutput")

    with tile.TileContext(nc) as tc:
        import contextlib

        ctx = contextlib.ExitStack()
        persist = ctx.enter_context(tc.tile_pool(name="persist", bufs=1))
        dramp = ctx.enter_context(tc.tile_pool(name="dramp", bufs=1, space="DRAM"))
        xw_pool = ctx.enter_context(tc.tile_pool(name="xw", bufs=2, space="PSUM"))
        acc_pool = ctx.enter_context(tc.tile_pool(name="acc", bufs=4, space="PSUM"))
        pacc_pool = ctx.enter_context(tc.tile_pool(name="pacc", bufs=1, space="PSUM"))
        sb_pool = ctx.enter_context(tc.tile_pool(name="work", bufs=3))
        msg_pool = ctx.enter_context(tc.tile_pool(name="msg", bufs=MSGB))
        oh_pool = ctx.enter_context(tc.tile_pool(name="oh", bufs=3))
        emb_pool = ctx.enter_context(tc.tile_pool(name="emb", bufs=3))
        craw = ctx.enter_context(tc.tile_pool(name="craw", bufs=1))

        def T(shape, dt, space=None, addr_space="Local", name=None):
            pool = dramp if space == "DRAM" else persist
            return pool.tile(shape, dt, tag=name, name=name, addr_space=addr_space)

        # ---- internal DRAM ----
        y_slice = [
            T([NPC, F], BF16, space="DRAM", name=f"y_slice{c}") for c in range(2)
        ]
        y_fq = [
            [T([RROWS, F], BF16, space="DRAM",
               addr_space="Local" if NOAG else "Shared", name=f"y_fq{c}_{q}")
             for q in range(4)]
            for c in range(2)
        ]
        dram_lin = T([G + 8, NCLS], F32, space="DRAM", name="dram_lin")
        ar_lin = T([G + 8, NCLS], F32, space="DRAM", addr_space="Shared",
                   name="ar_lin")

        # ---- persistent SBUF ----
        hA = T([P, NB * F], BF16, name="hA")  # conv2 output (node-major)
        hB = T([P, NB * F], BF16, name="hB")  # conv1 output (node-major)
        y_nm = T([P, NB * F], BF16, name="y_nm")
        dstc_sb = T([P, nch], BF16, name="dstc_sb")
        nc.sync.dma_start(out=dstc_sb[:], in_=dcd[:])
        eix_sb = T([P, totcol], I16, name="eix_sb")
        nc.sync.dma_start(out=eix_sb[:], in_=xixd[:])

        # constants
        iota_i = craw.tile([P, P], I32, tag="iota_i", name="iota_i")
        nc.gpsimd.iota(iota_i[:], pattern=[[1, P]], base=0, channel_multiplier=0)
        iota_bf = T([P, P], BF16, name="iota_bf")
        nc.vector.tensor_copy(iota_bf[:], iota_i[:])
        iota_f = T([P, P], F32, name="iota_f")
        nc.vector.tensor_copy(iota_f[:], iota_i[:])
        ones_row = T([1, P], F32, name="ones_row")
        nc.vector.memset(ones_row[:], 1.0)
        id_bf = T([P, P], BF16, name="id_bf")
        make_identity(nc, id_bf[:])

        def load_cast(name, dram, shape, dt_in, dt_out):
            t = T(shape, dt_out, name=name)
            if dt_out == dt_in:
                nc.sync.dma_start(out=t[:], in_=dram[:])
            else:
                # NB: SWDGE cast-DMA + indirect_dma in one program crashes the
                # device (observed NRT_EXEC_UNIT_UNRECOVERABLE) - cast on DVE.
                raw = craw.tile(shape, dt_in, tag=name + "_r", name=name + "_r")
                nc.sync.dma_start(out=raw[:], in_=dram[:])
                nc.vector.tensor_copy(t[:], raw[:])
            return t

        # tabw [3*P, F] in DRAM -> SBUF [P, 3*F] (plane-major in free dim)
        tabw_sb = T([P, 3 * F], BF16, name="tabw_sb")
        nc.sync.dma_start(
            out=tabw_sb[:].rearrange("p (c f) -> p c f", f=F),
            in_=tabwd[:].rearrange("(c p) f -> p c f", p=P),
        )
        W2c = load_cast("W2", W2d, [F, F], F32, BF16)
        bc = [
            load_cast("b1", b1d, [1, F], F32, F32),
            load_cast("b2", b2d, [1, F], F32, F32),
        ]
        Wl_sb = load_cast("Wl", Wld, [F, NCLS], F32, F32)
        bl_sb = load_cast("bl", bld, [1, NCLS], F32, F32)
        bl0_sb = load_cast("bl0", bl0d, [P, NB], F32, F32)
        bl1_sb = load_cast("bl1", bl1d, [P, NB], F32, F32)
        cnt_sb = load_cast("cnt", cntd, [P, NGT], F32, F32)
        gix_sb = load_cast("gix", gixd, [P, 2], I32, I32)

        # dinv = 1/sqrt(deg+1); sq = sqrt(deg+1)
        deg_sb = craw.tile([P, NB], F32, tag="deg_sb", name="deg_sb")
        nc.sync.dma_start(out=deg_sb[:], in_=degd[:])
        sq_sb = T([P, NB], F32, name="sq_sb")
        nc.scalar.sqrt(sq_sb[:], deg_sb[:])
        dinv = T([P, NB], F32, name="dinv")
        nc.vector.reciprocal(dinv[:], sq_sb[:])

        # blin broadcast down partitions (rank-1 via PE)
        blb_ps = xw_pool.tile([P, NCLS], F32, tag="xw")
        nc.tensor.matmul(blb_ps[:], lhsT=ones_row[:], rhs=bl_sb[:],
                         start=True, stop=True)
        blin_b = T([P, NCLS], F32, name="blin_b")
        nc.scalar.copy(blin_b[:], blb_ps[:])

        # zero dram_lin (pool scatter target) early
        zsb = craw.tile([P, NCLS * NGT], F32, tag="zsb", name="zsb")
        nc.vector.memset(zsb[:], 0.0)
        nc.sync.dma_start(
            out=dram_lin[:G, :].rearrange("(c p) f -> p c f", p=P),
            in_=zsb[:, : NGT * NCLS].rearrange("p (c f) -> p c f", f=NCLS),
        )
        nc.sync.dma_start(out=dram_lin[G:, :], in_=zsb[:8, :NCLS])

        def dma_node_rows(dst_dram, r0, r1):
            """DMA y_nm node rows [r0, r1) to dst_dram[r0:r1] (node-major)."""
            b0, o0 = divmod(r0, P)
            if o0:
                take = min(r1, (b0 + 1) * P) - r0
                nc.sync.dma_start(
                    out=dst_dram[r0 : r0 + take, :],
                    in_=y_nm[o0 : o0 + take, b0 * F : (b0 + 1) * F],
                )
                r0 += take
                b0 += 1
            nfull = (r1 - r0) // P
            if nfull:
                nc.sync.dma_start(
                    out=dst_dram[r0 : r0 + nfull * P, :].rearrange(
                        "(c p) f -> p c f", p=P),
                    in_=y_nm[:, b0 * F : (b0 + nfull) * F].rearrange(
                        "p (c f) -> p c f", f=F),
                )
                r0 += nfull * P
                b0 += nfull
            if r1 > r0:
                nc.sync.dma_start(
                    out=dst_dram[r0:r1, :],
                    in_=y_nm[: r1 - r0, b0 * F : (b0 + 1) * F],
                )

        def emit_ag_quarter(conv, q):
            r0, r1 = q * QR, (q + 1) * QR
            dma_node_rows(y_slice[conv], r0, r1)
            if NOAG:
                for kk in range(NCORES):
                    nc.sync.dma_start(
                        out=y_fq[conv][q][kk * QR : (kk + 1) * QR, :],
                        in_=y_slice[conv][r0:r1, :],
                    )
            else:
                nc.gpsimd.collective_compute(
                    "AllGather",
                    mybir.AluOpType.bypass,
                    replica_groups=RG,
                    ins=[y_slice[conv][r0:r1, :]],
                    outs=[y_fq[conv][q][:]],
                )

        # last block each src-quarter needs written before its AG can go
        QBMAX = [math.ceil((c + 1) * QR / P) - 1 for c in range(4)]

        # ---------------- embedding + xw1 fused -> y_nm ----------------
        ESC = math.ceil(NB / EGB)
        for s in range(ESC):
            t0 = s * EGB
            t1 = min(t0 + EGB, NB)
            ch = emb_pool.tile([P, EGB * 3 * F], BF16, tag="embch")
            nc.sync.dma_start(
                out=ch[:, : (t1 - t0) * 3 * F],
                in_=ohTd[:, t0 * 3 * F : t1 * 3 * F],
            )
            for t in range(t0, t1):
                base = (t - t0) * 3 * F
                xw = xw_pool.tile([P, P], F32, tag="xw")
                for j in range(3):
                    nc.tensor.matmul(
                        xw[:],
                        lhsT=ch[:, base + j * F : base + (j + 1) * F],
                        rhs=tabw_sb[:, j * F : (j + 1) * F],
                        start=(j == 0),
                        stop=(j == 2),
                    )
                nc.scalar.activation(
                    y_nm[:, t * F : (t + 1) * F],
                    xw[:],
                    mybir.ActivationFunctionType.Copy,
                    scale=dinv[:, t : t + 1],
                )
            for c in range(4):
                if t0 <= QBMAX[c] < t1:
                    emit_ag_quarter(0, c)

        # ---------------- two GCN convs ----------------
        def make_b_bcast(conv):
            # b_bcast[n, f] = b[f] replicated down partitions (rank-1 via PE)
            bb_ps = xw_pool.tile([P, P], F32, tag="xw")
            nc.tensor.matmul(bb_ps[:], lhsT=ones_row[:], rhs=bc[conv][:],
                             start=True, stop=True)
            b_bcast = craw.tile([P, P], F32, tag=f"b_bcast{conv}", name=f"b_bcast{conv}")
            nc.scalar.copy(b_bcast[:], bb_ps[:])
            return b_bcast

        def emit_xw2_tiles(tlo, thi):
            """y2 = dinv * (h1 @ W2) for tiles [tlo, thi)."""
            for t in range(tlo, thi):
                hT = sb_pool.tile([P, P], BF16, tag="hT")
                if USE_DMAT:
                    nc.sync.dma_start_transpose(
                        hT[:], hB[:, t * F : (t + 1) * F])
                else:
                    tp = xw_pool.tile([P, P], BF16, tag="xw")
                    nc.tensor.transpose(tp[:], hB[:, t * F : (t + 1) * F],
                                        id_bf[:])
                    nc.scalar.copy(hT[:], tp[:])
                xw = xw_pool.tile([P, P], F32, tag="xw")
                nc.tensor.matmul(xw[:], lhsT=hT[:], rhs=W2c[:],
                                 start=True, stop=True)
                nc.scalar.activation(
                    y_nm[:, t * F : (t + 1) * F],
                    xw[:],
                    mybir.ActivationFunctionType.Copy,
                    scale=dinv[:, t : t + 1],
                )

        do_pool = PHASES >= 9
        pacc = [pacc_pool.tile([P, P], F32, tag=f"pacc{h}", name=f"pacc{h}")
                for h in range(2)]
        bls = [bl0_sb, bl1_sb]

        def emit_pool_block(t):
            # paccT[h][f, g] += h2[n, f]^T @ onehot(graph[n] == g_h)
            for h in range(2):
                oht = sb_pool.tile([P, P], BF16, tag="pooloh")
                nc.vector.tensor_tensor(
                    out=oht[:],
                    in0=iota_f[:],
                    in1=bls[h][:, t : t + 1].to_broadcast([P, P]),
                    op=mybir.AluOpType.is_equal,
                )
                nc.tensor.matmul(
                    pacc[h][:],
                    lhsT=hA[:, t * F : (t + 1) * F],
                    rhs=oht[:],
                    start=(t == 0),
                    stop=(t == NB - 1),
                )

        def emit_scatter(conv, hout, b_bcast, after_group=None, after_block=None):
            ch_off = 0  # chunk offset into dst_cols
            col_off = [0]  # column offset into edge_idx
            for gi, blocks in enumerate(groups if SUB >= 2 else []):
                nblk = len(blocks)
                nch_q = nblk * Cq
                nidx = nch_q * P
                msgs = []
                for q in range(4):
                    msg = msg_pool.tile([P, GB * Cq, F], BF16, tag="msg")
                    nc.gpsimd.dma_gather(
                        out_ap=msg[:, :nch_q, :],
                        in_ap=y_fq[conv][q][:, :],
                        idxs_ap=eix_sb[:, col_off[0] : col_off[0] + nidx // 16],
                        num_idxs=nidx,
                        num_idxs_reg=nidx,
                        elem_size=F,
                        single_packet=SP,
                        queue_num=q,
                    )
                    col_off[0] += nidx // 16
                    msgs.append(msg)
                if SUB < 3:
                    ch_off += 4 * nblk * Cq
                    continue
                for bi, b in enumerate(blocks):
                    oh = oh_pool.tile([P, 4 * Cq, P], BF16, tag="oh")
                    nc.vector.tensor_tensor(
                        out=oh[:],
                        in0=iota_bf[:].unsqueeze(1).broadcast_to([P, 4 * Cq, P]),
                        in1=dstc_sb[:, ch_off + bi * 4 * Cq : ch_off + (bi + 1) * 4 * Cq]
                        .unsqueeze(2)
                        .broadcast_to([P, 4 * Cq, P]),
                        op=mybir.AluOpType.is_equal,
                    )
                    if SUB < 4:
                        continue
                    acc = acc_pool.tile([P, P], F32, tag="acc")
                    j = 0
                    for q in range(4):
                        for cc in range(Cq):
                            nc.tensor.matmul(
                                acc[:],
                                lhsT=oh[:, q * Cq + cc, :],
                                rhs=msgs[q][:, bi * Cq + cc, :],
                                start=(j == 0),
                                stop=(j == 4 * Cq - 1),
                            )
                            j += 1
                    # bias: bb = b (x) rdinv (cancels the later *dinv); ACT op
                    bb = sb_pool.tile([P, P], F32, tag="bb")
                    nc.scalar.activation(
                        bb[:], b_bcast[:], mybir.ActivationFunctionType.Copy,
                        scale=sq_sb[:, b : b + 1],
                    )
                    hs = sb_pool.tile([P, P], F32, tag="ep")
                    nc.vector.tensor_tensor(
                        out=hs[:],
                        in0=acc[:],
                        in1=y_nm[:, b * F : (b + 1) * F],
                        op=mybir.AluOpType.add,
                    )
                    nc.vector.tensor_tensor(
                        out=hs[:], in0=hs[:], in1=bb[:], op=mybir.AluOpType.add,
                    )
                    nc.vector.tensor_tensor(
                        out=hs[:],
                        in0=hs[:],
                        in1=dinv[:, b : b + 1].to_broadcast([P, P]),
                        op=mybir.AluOpType.mult,
                    )
                    nc.scalar.activation(
                        hout[:, b * F : (b + 1) * F],
                        hs[:],
                        mybir.ActivationFunctionType.Relu,
                    )
                    if after_block is not None:
                        after_block(b)
                ch_off += 4 * nch_q
                if after_group is not None:
                    after_group(blocks[-1])

        # conv1 scatter, with xw2 tiles + conv2 quarter-AGs interleaved as
        # each src-quarter's h1 blocks complete
        nconv = min(2, max(0, PHASES - 1))
        if nconv >= 1:
            b_bc0 = make_b_bcast(0)
            if nconv >= 2:
                ag_state = [0]  # next quarter to emit

                def after_group1(maxb):
                    while ag_state[0] < 4 and QBMAX[ag_state[0]] <= maxb:
                        c = ag_state[0]
                        tlo = 0 if c == 0 else QBMAX[c - 1] + 1
                        emit_xw2_tiles(tlo, QBMAX[c] + 1)
                        emit_ag_quarter(1, c)
                        ag_state[0] += 1
            else:
                after_group1 = None
            emit_scatter(0, hB, b_bc0, after_group=after_group1)
        # conv2 scatter, with pool matmuls interleaved per finished block
        if nconv >= 2:
            b_bc1 = make_b_bcast(1)
            emit_scatter(1, hA, b_bc1,
                         after_block=emit_pool_block if do_pool else None)

        # ---------------- pool epilogue: linear, AllReduce, mean ----------------
        if do_pool:
            for h in range(2):
                pT = sb_pool.tile([P, P], F32, tag="pT")
                nc.vector.tensor_copy(pT[:], pacc[h][:])
                lin_ps = xw_pool.tile([P, NCLS], F32, tag="xw")
                nc.tensor.matmul(lin_ps[:], lhsT=pT[:], rhs=Wl_sb[:],
                                 start=True, stop=True)
                linc = sb_pool.tile([P, NCLS], F32, tag="linc")
                nc.vector.tensor_copy(linc[:], lin_ps[:])
                nc.gpsimd.indirect_dma_start(
                    out=dram_lin[:],
                    out_offset=bass.IndirectOffsetOnAxis(ap=gix_sb[:, h : h + 1], axis=0),
                    in_=linc[:],
                    in_offset=None,
                )
            nc.gpsimd.collective_compute(
                "AllReduce",
                mybir.AluOpType.add,
                replica_groups=RG,
                ins=[dram_lin[:]],
                outs=[ar_lin[:]],
            )
            recip = T([P, NGT], F32, name="recip")
            nc.vector.reciprocal(recip[:], cnt_sb[:])
            for t in range(NGT):
                art = sb_pool.tile([P, NCLS], F32, tag="art")
                nc.sync.dma_start(out=art[:], in_=ar_lin[t * P : (t + 1) * P, :])
                pooled = sb_pool.tile([P, NCLS], F32, tag="linc")
                nc.vector.tensor_tensor(
                    out=pooled[:],
                    in0=art[:],
                    in1=recip[:, t : t + 1].to_broadcast([P, NCLS]),
                    op=mybir.AluOpType.mult,
                )
                oute = sb_pool.tile([P, NCLS], F32, tag="oute")
                nc.vector.tensor_tensor(
                    out=oute[:], in0=pooled[:], in1=blin_b[:],
                    op=mybir.AluOpType.add,
                )
                nc.sync.dma_start(out=outd[t * P : (t + 1) * P, :], in_=oute[:])

        else:
            dummy = sb_pool.tile([P, NCLS], F32, tag="oute", name="dummy")
            nc.vector.memset(dummy[:], 0.0)
            for t in range(NGT):
                nc.sync.dma_start(out=outd[t * P : (t + 1) * P, :], in_=dummy[:])

        ctx.close()
    nc.compile()
    return nc


_CACHE = {}


def _get_nc(cfg, Cq, pl, totcol, nch):
    key = (tuple(sorted(cfg.items())), Cq, totcol, nch, GB, MSGB, EGB, SP, USE_DMAT)
    if key not in _CACHE:
        _CACHE[key] = _build(cfg, Cq, pl, totcol, nch)
    return _CACHE[key]


def run(inputs, cfg, trace=False):
    x = np.asarray(inputs["x"])
    per_core, Cq, pl = _prep_host(x, np.asarray(inputs["edge_index"]),
                                  np.asarray(inputs["batch"]), cfg, inputs)
    totcol = per_core[0]["edge_idx"].shape[1]
    nch = per_core[0]["dst_cols"].shape[1]
    nc = _get_nc(cfg, Cq, pl, totcol, nch)

    shared = dict(
        W2=np.asarray(inputs["W2"], np.float32),
        b1=np.asarray(inputs["b1"], np.float32).reshape(1, -1),
        b2=np.asarray(inputs["b2"], np.float32).reshape(1, -1),
        Wlin=np.asarray(inputs["Wlin"], np.float32),
        blin=np.asarray(inputs["blin"], np.float32).reshape(1, -1),
    )
    in_maps = [{**shared, **per_core[k]} for k in range(NCORES)]
    res = run_bass_kernel_spmd(nc, in_maps, list(range(NCORES)), trace=trace)
    out = np.asarray(res.results[0]["out"], np.float32)
    return out, res


def kernel(**inputs) -> np.ndarray:
    out, _ = run(inputs, CFG)
    return out
